# revision 1
# baseline (speedup 1.0000x reference)
"""Trainium2 Bass kernel for nn_CpSae_44014824849572.

Computes the CP-SAE loss. The reference materializes a [1024, 64, 32, 32]
CP-reconstruction `volume` and diffs it against `features`. We instead use

  sum((flat - volume)^2) = sum(flat^2) - 2*sum(flat*volume) + sum(volume^2)

with  sum(flat*volume)[b] = sum_z a[b,z] * T[b,z],
      T[b,z]   = sum_feat flat[b,feat] * KRP[g_b][z,feat]
      KRP[g]   = softplus(freq)⊗softplus(roi1)⊗softplus(roi2)  (rank-1 rows)
      sum(volume^2)[b] = a_b^T M_{g_b} a_b,
      M_g = (Ff Ff^T) ∘ (R1 R1^T) ∘ (R2 R2^T)   (32x32 per group, tiny)

so the only heavy device work is two big contractions over the feature dim:
  zcat[b, 0:64] = flat[b] @ [W1 | W2]          (encoder, 8.6 GFLOP)
  T[b, z]       = flat[b] @ KRP[g_b].T         (4.3 GFLOP)

Distribution: feature-dim sharded across 8 cores (8192 features each, all
1024 samples -> moving free dim of 512 per matmul). Samples are sorted by
group on the host so each group's T-matmul sees a contiguous column block
with one shared stationary operand; groups are packed 4-at-a-time into the
128 PE columns. Encoder matmuls for even/odd k-chunks run concurrently on
disjoint PE column halves via tile_position. Data is fp8e4m3 (weights
pre-scaled) with fp32 PSUM accumulation; partial zcat/T are summed on host.
"""
import json

import numpy as np
import ml_dtypes

import concourse.bass as bass
import concourse.mybir as mybir
import concourse.tile as tile
from concourse.bass_utils import run_bass_kernel_spmd

N_CORES = 8
BATCH = 1024
N_FREQS = 64
N_ROIS = 32
Z = 32
N_GROUPS = 16
N_CLASSES = 4
N_FEAT = N_FREQS * N_ROIS * N_ROIS          # 65536
FEAT_PER_CORE = N_FEAT // N_CORES           # 8192
KCHUNKS = FEAT_PER_CORE // 128              # 64
CHUNKS_PER_DMA = 2
KRP_SLICES = 8
EPSILON = 1e-06
REG_STRENGTH = 1.0
KL_FACTOR = 1.0

F32 = mybir.dt.float32
DATA_MODE = "fp8"                           # "fp8" | "bf16"
if DATA_MODE == "fp8":
    DT = mybir.dt.float8e4
    NPDT = ml_dtypes.float8_e4m3
    W_SCALE = 4096.0
else:
    DT = mybir.dt.bfloat16
    NPDT = ml_dtypes.bfloat16
    W_SCALE = 1.0
NPBYTES = np.dtype(NPDT).itemsize
_U = np.uint8 if NPBYTES == 1 else np.uint16

_waitfix_counter = [0]


def _split_waits_in_bir(bir: dict) -> int:
    """This container's walrus accepts only ONE sync wait per instruction;
    Tile emits several. Hoist all-but-one wait onto EventSemaphore
    instructions inserted just before, on the same engine."""
    nsplit = 0
    for fn in bir.get("functions", []):
        for blk in fn.get("blocks", []):
            out = []
            for insn in blk.get("instructions", []):
                si = insn.get("sync_info") or {}
                ow = si.get("on_wait") or []
                if len(ow) > 1:
                    for w in ow[:-1]:
                        _waitfix_counter[0] += 1
                        out.append({
                            "debug": insn.get("debug", 0),
                            "engine": insn["engine"],
                            "ins": [],
                            "name": f"{insn['name']}-wsplit{_waitfix_counter[0]}",
                            "opcode": "EventSemaphore",
                            "outs": [],
                            "sync_info": {"on_update": [], "on_wait": [w]},
                        })
                        nsplit += 1
                    si["on_wait"] = [ow[-1]]
                out.append(insn)
            blk["instructions"] = out
    return nsplit


def _install_waitfix():
    import concourse.bass2jax as bass2jax
    import concourse.bass_utils as bass_utils

    if getattr(bass2jax, "_waitfix_installed", False):
        return
    orig = bass_utils.compile_bir_kernel

    def patched(bir_json, tmpdir, neff_name="file.neff"):
        bir = json.loads(bir_json.decode() if isinstance(bir_json, bytes) else bir_json)
        _split_waits_in_bir(bir)
        return orig(json.dumps(bir).encode(), tmpdir, neff_name)

    bass2jax.compile_bir_kernel = patched
    bass_utils.compile_bir_kernel = patched
    bass2jax._waitfix_installed = True


def _softplus(x):
    return np.logaddexp(0.0, x.astype(np.float64)).astype(np.float32)


def _quartet_blocks(groups_sorted):
    """[(q, c0, c1)] contiguous column blocks (<=512 wide) per group-quartet
    q (groups 4q..4q+3)."""
    gs = np.asarray(groups_sorted)
    blocks = []
    for q in range(N_GROUPS // 4):
        c0 = int(np.searchsorted(gs, 4 * q))
        c1 = int(np.searchsorted(gs, 4 * q + 4))
        while c0 < c1:
            ce = min(c0 + 512, c1)
            blocks.append((q, c0, ce))
            c0 = ce
    return blocks


def build_device_program(blocks, parts="all"):
    """One SPMD program (shared by all 8 cores). Per-core inputs:
      flatt [KCHUNKS, 128, BATCH]  — transposed feature slice (group-sorted)
      w     [128, KCHUNKS, 64]     — [W1|W2]*W_SCALE slice, partition-major
      krpt  [128, KCHUNKS, 16, Z]  — KRP slice, partition-major
    Outputs (partial sums over this core's features):
      zcat [128, BATCH] f32 — rows 0:64 even-k half, 64:128 odd-k half
      tt   [128, BATCH] f32 — row (g%4)*32+z holds T[z] for that column's group
    """
    nc = bass.Bass()
    flatt = nc.dram_tensor("flatt", [KCHUNKS, 128, BATCH], DT, kind="ExternalInput")
    w = nc.dram_tensor("w", [128, KCHUNKS, 64], DT, kind="ExternalInput")
    krpt = nc.dram_tensor("krpt", [128, KCHUNKS, N_GROUPS, Z], DT, kind="ExternalInput")
    zcat_out = nc.dram_tensor("zcat", [128, BATCH], F32, kind="ExternalOutput")
    tt_out = nc.dram_tensor("tt", [128, BATCH], F32, kind="ExternalOutput")

    kc_per_slice = KCHUNKS // KRP_SLICES

    with tile.TileContext(nc) as tc:
        with (
            tc.tile_pool(name="fpool", bufs=6) as fpool,
            tc.tile_pool(name="const", bufs=1) as const,
            tc.tile_pool(name="opool", bufs=1) as opool,
            tc.tile_pool(name="psum", bufs=1, space="PSUM") as psum,
        ):
            wt = const.tile([128, KCHUNKS, 64], DT, tag="w")
            if parts != "pe":
                nc.sync.dma_start(out=wt, in_=w[:, :, :])
            krp_tiles = []
            for j in range(KRP_SLICES):
                kt = const.tile([128, kc_per_slice, N_GROUPS, Z], DT, tag=f"krp{j}")
                if parts != "pe":
                    nc.sync.dma_start(
                        out=kt,
                        in_=krpt[:, j * kc_per_slice:(j + 1) * kc_per_slice, :, :],
                    )
                krp_tiles.append(kt)

            zcat_ps = t_ps = None
            if parts != "dma":
                zcat_ps = psum.tile([128, BATCH], F32, tag="zcat")
                t_ps = psum.tile([128, BATCH], F32, tag="t")

            for k0 in range(0, KCHUNKS, CHUNKS_PER_DMA):
                nch = min(CHUNKS_PER_DMA, KCHUNKS - k0)
                ft = fpool.tile([128, nch, BATCH], DT, tag="flat")
                if parts != "pe":
                    nc.sync.dma_start(
                        out=ft, in_=flatt[k0:k0 + nch, :, :].rearrange("c p n -> p c n")
                    )
                for kk in range(nch):
                    k = k0 + kk
                    if parts == "dma":
                        continue
                    start = k < 2
                    stop = k >= KCHUNKS - 2
                    par = k % 2
                    # encoder: even/odd k-chunks on disjoint PE column halves
                    for half in range(2):
                        nc.tensor.matmul(
                            zcat_ps[par * 64:(par + 1) * 64,
                                    half * 512:(half + 1) * 512],
                            wt[:, k, :],
                            ft[:, kk, half * 512:(half + 1) * 512],
                            start=start,
                            stop=stop,
                            tile_position=(0, par * 64),
                        )
                    # T: 4 groups packed into the 128 PE columns per matmul
                    kt = krp_tiles[k // kc_per_slice]
                    kloc = k % kc_per_slice
                    for (q, c0, c1) in blocks:
                        nc.tensor.matmul(
                            t_ps[:, c0:c1],
                            kt[:, kloc, 4 * q:4 * (q + 1), :],
                            ft[:, kk, c0:c1],
                            start=(k == 0),
                            stop=(k == KCHUNKS - 1),
                        )

            if parts != "dma":
                zc_sb = opool.tile([128, BATCH], F32, tag="zc")
                nc.vector.tensor_copy(zc_sb, zcat_ps)
                nc.sync.dma_start(out=zcat_out[:, :], in_=zc_sb)
                tt_sb = opool.tile([128, BATCH], F32, tag="tt")
                nc.scalar.copy(tt_sb, t_ps)
                nc.sync.dma_start(out=tt_out[:, :], in_=tt_sb)
    return nc


def _prepare(inputs):
    features = np.asarray(inputs["features"], dtype=np.float32)
    labels = np.asarray(inputs["labels"]).astype(np.int64)
    groups = np.asarray(inputs["groups"]).astype(np.int64)
    weights = np.asarray(inputs["weights"], dtype=np.float32)
    noise = np.asarray(inputs["noise"], dtype=np.float32)
    group_embed = np.asarray(inputs["group_embed"], dtype=np.float32)
    W1 = np.asarray(inputs["W1"], dtype=np.float32)
    b1 = np.asarray(inputs["b1"], dtype=np.float32)
    W2 = np.asarray(inputs["W2"], dtype=np.float32)
    b2 = np.asarray(inputs["b2"], dtype=np.float32)
    freq_factors = np.asarray(inputs["freq_factors"], dtype=np.float32)
    roi_1_factors = np.asarray(inputs["roi_1_factors"], dtype=np.float32)
    roi_2_factors = np.asarray(inputs["roi_2_factors"], dtype=np.float32)
    lin_W = np.asarray(inputs["lin_W"], dtype=np.float32)
    lin_b = np.asarray(inputs["lin_b"], dtype=np.float32)
    logit_bias = np.asarray(inputs["logit_bias"], dtype=np.float32)

    b = features.shape[0]
    flat = features.reshape(b, -1)

    perm = np.argsort(groups, kind="stable")
    groups_sorted = groups[perm]
    blocks = _quartet_blocks(groups_sorted)

    sq = np.einsum("bi,bi->b", flat, flat, optimize=True)

    flat_q = flat[perm].astype(NPDT)
    flatT = flat_q.view(_U).T.copy().view(NPDT)            # [N_FEAT, BATCH]

    W = (np.concatenate([W1[:N_FEAT], W2[:N_FEAT]], axis=1) * W_SCALE).astype(NPDT)

    Ff = _softplus(freq_factors)
    R1 = _softplus(roi_1_factors)
    R2 = _softplus(roi_2_factors)

    krp = np.empty((N_GROUPS, Z, N_FEAT), dtype=NPDT)
    for g in range(N_GROUPS):
        r12 = np.einsum("zr,zs->zrs", R1[g], R2[g]).reshape(Z, N_ROIS * N_ROIS)
        krp[g] = np.einsum("zf,zx->zfx", Ff[g], r12).reshape(Z, N_FEAT).astype(NPDT)
    krpt = krp.view(_U).transpose(2, 0, 1).reshape(N_CORES, KCHUNKS, 128, N_GROUPS, Z)
    krpt = krpt.transpose(0, 2, 1, 3, 4).copy().view(NPDT)

    w_dev = W.view(_U).reshape(N_CORES, KCHUNKS, 128, 64)
    w_dev = w_dev.transpose(0, 2, 1, 3).copy().view(NPDT)

    in_maps = []
    for c in range(N_CORES):
        in_maps.append({
            "flatt": np.ascontiguousarray(
                flatT[c * FEAT_PER_CORE:(c + 1) * FEAT_PER_CORE].view(_U)
            ).reshape(KCHUNKS, 128, BATCH).view(NPDT),
            "w": w_dev[c],
            "krpt": krpt[c],
        })

    host = dict(
        labels=labels, groups=groups, weights=weights, noise=noise,
        group_embed=group_embed, W1=W1, b1=b1, W2=W2, b2=b2,
        lin_W=lin_W, lin_b=lin_b, logit_bias=logit_bias,
        Ff=Ff, R1=R1, R2=R2, sq=sq, perm=perm, b=b,
        groups_sorted=groups_sorted,
    )
    return in_maps, blocks, host


def _finish(zcatT, ttT, host):
    b = host["b"]
    perm = host["perm"]
    inv = np.empty_like(perm)
    inv[perm] = np.arange(b)

    zcat = ((zcatT[:64] + zcatT[64:]) / W_SCALE).T[inv]    # [b, 64]
    gs4 = (host["groups_sorted"] % 4).astype(np.int64)     # row quartet per sorted col
    cols = np.arange(b)
    T_sorted = np.empty((b, Z), np.float32)
    for zi in range(Z):
        T_sorted[:, zi] = ttT[gs4 * Z + zi, cols]
    T = T_sorted[inv]

    groups = host["groups"]
    ge = host["group_embed"][groups]
    z_mu = zcat[:, :Z] + host["b1"] + ge @ host["W1"][N_FEAT:]
    z_log_std = zcat[:, Z:] + host["b2"] + ge @ host["W2"][N_FEAT:]
    sigma = EPSILON + np.exp(z_log_std)
    kld = np.sum(-np.log(sigma) + 0.5 * (sigma * sigma + z_mu * z_mu - 1.0), axis=1)
    zs = z_mu + sigma * host["noise"]
    zs = zs @ host["lin_W"] + host["lin_b"]
    a = _softplus(zs)

    Ff, R1, R2 = host["Ff"], host["R1"], host["R2"]
    M = (np.einsum("gzf,gyf->gzy", Ff, Ff)
         * np.einsum("gzr,gyr->gzy", R1, R1)
         * np.einsum("gzs,gys->gzy", R2, R2))
    vol2 = np.einsum("bz,bzy,by->b", a, M[groups], a)
    fdotv = np.sum(a * T, axis=1)
    rec = REG_STRENGTH * (host["sq"] - 2.0 * fdotv + vol2) / N_FEAT

    logits = np.concatenate([zs[:, :N_CLASSES - 1], np.ones((b, 1), np.float32)],
                            axis=1) + host["logit_bias"]
    m = logits.max(axis=1, keepdims=True)
    lse = m[:, 0] + np.log(np.exp(logits - m).sum(axis=1))
    log_probs = logits[np.arange(b), host["labels"]] - lse

    freq_loss = np.var(Ff, axis=0, ddof=1).mean(axis=1).sum()
    roi_loss = (np.var(R1, axis=0, ddof=1) + np.var(R2, axis=0, ddof=1)).mean(axis=1).sum()

    loss = np.mean(rec - host["weights"] * log_probs + KL_FACTOR * kld) \
        + freq_loss + roi_loss
    return np.float32(loss)


def kernel(**inputs) -> np.ndarray:
    _install_waitfix()
    in_maps, blocks, host = _prepare(inputs)
    nc = build_device_program(blocks)
    r = run_bass_kernel_spmd(nc, in_maps, core_ids=list(range(N_CORES)))
    zcatT = np.zeros((128, BATCH), np.float32)
    ttT = np.zeros((128, BATCH), np.float32)
    for c in range(N_CORES):
        zcatT += r.results[c]["zcat"]
        ttT += r.results[c]["tt"]
    return _finish(zcatT, ttT, host)



# revision 29
# speedup vs baseline: 2.3001x; 2.3001x over previous
"""Trainium2 Bass kernel for nn_CpSae_44014824849572.

Computes the CP-SAE loss. The reference materializes a [1024, 64, 32, 32]
CP-reconstruction `volume` and diffs it against `features`. We instead use

  sum((flat - volume)^2) = sum(flat^2) - 2*sum(flat*volume) + sum(volume^2)

with  sum(flat*volume)[b] = sum_z a[b,z] * T[b,z],
      T[b,z]   = sum_feat flat[b,feat] * KRP[g_b][z,feat]
      KRP[g]   = softplus(freq)⊗softplus(roi1)⊗softplus(roi2)  (rank-1 rows)
      sum(volume^2)[b] = a_b^T M_{g_b} a_b,
      M_g = (Ff Ff^T) ∘ (R1 R1^T) ∘ (R2 R2^T)   (32x32 per group, tiny)

so the only heavy device work is two big contractions over the feature dim:
  zcat[b, 0:64] = flat[b] @ [W1 | W2]          (encoder, 8.6 GFLOP)
  T[b, z]       = flat[b] @ KRP[g_b].T         (4.3 GFLOP)

Distribution: feature-dim sharded across 8 cores (8192 features each = 8
frequency indices x 1024 roi-pairs). All matmuls run in fp8 DoubleRow mode
(256-deep contraction, 0.5 PE cycles per output column). The KRP stationary
is factored: KRP[g][z, (f,r,s)] = Ff[g,z,f] * R12[g,z,(r,s)]; the device
contracts against R12 only (shared across the core's 8 f-indices, 512 KB)
accumulating per-f partial sums P_f in PSUM, then folds the Ff factor in
during the PSUM drain with a fused per-partition multiply-add on DVE:
T += Ff[f,:] * P_f. This keeps total input DMA to ~9 MB/core (the DMA
engines are the bottleneck; flat alone is 8 MB). Samples are host-sorted by
group so each group-pair's T columns are contiguous; outputs return as bf16
partial sums that the host reduces across cores.
"""
import json

import numpy as np
import ml_dtypes

import concourse.bass as bass
import concourse.mybir as mybir
import concourse.tile as tile
from concourse.bass_utils import run_bass_kernel_spmd

N_CORES = 8
BATCH = 1024
N_FREQS = 64
N_ROIS = 32
Z = 32
N_GROUPS = 16
N_CLASSES = 4
N_FEAT = N_FREQS * N_ROIS * N_ROIS          # 65536
FEAT_PER_CORE = N_FEAT // N_CORES           # 8192
KPAIRS = FEAT_PER_CORE // 256               # 32 double-chunks of 256 features
F_PER_CORE = N_FREQS // N_CORES             # 8 frequency indices per core
RS = N_ROIS * N_ROIS                        # 1024 roi-pair features per f
KP_PER_CHUNK = 1                            # flat DMA granularity (256 KB)
NCHUNKS = KPAIRS // KP_PER_CHUNK
N_TPS = 3                                   # T psum rotation depth
EPSILON = 1e-06
REG_STRENGTH = 1.0
KL_FACTOR = 1.0

F32 = mybir.dt.float32
BF16 = mybir.dt.bfloat16
FP8 = mybir.dt.float8e4
NPFP8 = ml_dtypes.float8_e4m3
NPBF16 = ml_dtypes.bfloat16
W_SCALE = 4096.0
DR = mybir.MatmulPerfMode.DoubleRow
MULT = mybir.AluOpType.mult
ADD = mybir.AluOpType.add

_waitfix_counter = [0]


def _split_waits_in_bir(bir: dict) -> int:
    """This container's walrus accepts only ONE sync wait per instruction;
    Tile emits several. Hoist all-but-one wait onto EventSemaphore
    instructions inserted just before, on the same engine."""
    nsplit = 0
    for fn in bir.get("functions", []):
        for blk in fn.get("blocks", []):
            out = []
            for insn in blk.get("instructions", []):
                si = insn.get("sync_info") or {}
                ow = si.get("on_wait") or []
                if len(ow) > 1:
                    for w in ow[:-1]:
                        _waitfix_counter[0] += 1
                        out.append({
                            "debug": insn.get("debug", 0),
                            "engine": insn["engine"],
                            "ins": [],
                            "name": f"{insn['name']}-wsplit{_waitfix_counter[0]}",
                            "opcode": "EventSemaphore",
                            "outs": [],
                            "sync_info": {"on_update": [], "on_wait": [w]},
                        })
                        nsplit += 1
                    si["on_wait"] = [ow[-1]]
                out.append(insn)
            blk["instructions"] = out
    return nsplit


def _install_waitfix():
    import concourse.bass2jax as bass2jax
    import concourse.bass_utils as bass_utils

    if getattr(bass2jax, "_waitfix_installed", False):
        return
    orig = bass_utils.compile_bir_kernel

    def patched(bir_json, tmpdir, neff_name="file.neff"):
        bir = json.loads(bir_json.decode() if isinstance(bir_json, bytes) else bir_json)
        _split_waits_in_bir(bir)
        return orig(json.dumps(bir).encode(), tmpdir, neff_name)

    bass2jax.compile_bir_kernel = patched
    bass_utils.compile_bir_kernel = patched
    bass2jax._waitfix_installed = True


def _softplus(x):
    return np.logaddexp(0.0, x.astype(np.float64)).astype(np.float32)


def _pair_blocks(groups_sorted):
    """[(q, c0, c1)] contiguous column blocks (<=256 wide) per group-pair q
    (groups 2q, 2q+1). Blocks never straddle column 512: columns 0:512 are
    drained by DVE, 512:1024 by Pool, and each lives in its own psum tile."""
    gs = np.asarray(groups_sorted)
    blocks = []
    for q in range(N_GROUPS // 2):
        c0 = int(np.searchsorted(gs, 2 * q))
        c1 = int(np.searchsorted(gs, 2 * q + 2))
        while c0 < c1:
            ce = min(c0 + 256, c1)
            if c0 < 512 < ce:
                ce = 512
            blocks.append((q, c0, ce))
            c0 = ce
    return blocks


def build_device_program(blocks):
    """One SPMD program (shared by all 8 cores). Per-core inputs:
      flatt [128, KPAIRS, 2, BATCH] fp8 — feature slice, group-sorted columns;
                                          feature = kp*256 + sub*128 + p
      w     [128, KPAIRS, 2, 64]   fp8 — [W1|W2]*W_SCALE slice
      r12   [128, 4, 2, 512]       fp8 — R12*R_SCALE, rs = blk*256+sub*128+p,
                                          free axis is gz (group-major)
      ff    [128, 8, 8]            f32 — Ff/R_SCALE per-partition scale
                                          vectors: [i, f, q] with
                                          g = 2q+i//32, z = i%32 (i < 64)
    Outputs (partial sums over this core's features, bf16):
      zcat [64, BATCH] — encoder output [W1|W2] rows
      tt   [64, BATCH] — row 32*(g%2)+z holds T[z] for that column's group
    """
    nc = bass.Bass()
    flatt = nc.dram_tensor("flatt", [128, KPAIRS, 2, BATCH], FP8, kind="ExternalInput")
    w = nc.dram_tensor("w", [128, KPAIRS, 2, 64], FP8, kind="ExternalInput")
    r12 = nc.dram_tensor("r12", [128, 4, 2, 512], FP8, kind="ExternalInput")
    ff = nc.dram_tensor("ff", [128, F_PER_CORE, 8], F32, kind="ExternalInput")
    zcat_out = nc.dram_tensor("zcat", [64, BATCH], BF16, kind="ExternalOutput")
    tt_out = nc.dram_tensor("tt", [64, BATCH], BF16, kind="ExternalOutput")

    # Drain-engine column split: DVE owns columns 0:512, Pool owns 512:1024.
    cb = 512

    with tile.TileContext(nc) as tc:
        with (
            tc.tile_pool(name="fpool", bufs=1) as fpool,
            tc.tile_pool(name="const", bufs=1) as const,
            tc.tile_pool(name="psum", bufs=1, space="PSUM") as psum,
        ):
            # --- input DMAs ---
            # sync(SP) SEQ carries the flat stream; scalar(Act) SEQ carries
            # the small tensors (its first transfers interleave after flat
            # chunk 0, so PE can start the encoder almost immediately).
            flat_tiles = []
            wt = const.tile([128, KPAIRS, 2, 64], FP8, tag="w")
            r12t = const.tile([128, 4, 2, 512], FP8, tag="r12")
            # one fft copy per drain engine: a shared tile would serialize
            # DVE and Pool through the dep tracker's single access chain
            fft_v = const.tile([128, F_PER_CORE, 8], F32, tag="ffv")
            fft_p = const.tile([128, F_PER_CORE, 8], F32, tag="ffp")
            # flat_tiles[kp] = [(col_off, tile), ...]; the last kps arrive as
            # two column-half DMAs so PE can start on the first half while
            # the second is still in flight (blocks never straddle col 512).
            SPLIT_LAST = 2
            for kp in range(KPAIRS):
                if kp < KPAIRS - SPLIT_LAST:
                    t = fpool.tile([128, 1, 2, BATCH], FP8, tag=f"flat{kp}",
                                   name=f"flat{kp}")
                    flat_tiles.append([(0, t)])
                    nc.sync.dma_start(out=t, in_=flatt[:, kp:kp + 1, :, :])
                else:
                    parts = []
                    for hi, off in enumerate((0, 512)):
                        t = fpool.tile([128, 1, 2, 512], FP8, tag=f"flat{kp}_{hi}",
                                       name=f"flat{kp}_{hi}")
                        parts.append((off, t))
                        nc.sync.dma_start(
                            out=t, in_=flatt[:, kp:kp + 1, :, off:off + 512])
                    flat_tiles.append(parts)
                if kp == 0:
                    nc.scalar.dma_start(out=wt, in_=w[:, :, :, :])
                    nc.scalar.dma_start(out=r12t, in_=r12[:, :, :, :])
                    nc.scalar.dma_start(out=fft_v, in_=ff[:, :, :])
                    nc.scalar.dma_start(out=fft_p, in_=ff[:, :, :])

            def ft_rhs(kp, c0, c1):
                for off, t in flat_tiles[kp]:
                    w_ = t.shape[3]
                    if off <= c0 and c1 <= off + w_:
                        return t[:, 0, :, c0 - off:c1 - off]
                raise AssertionError((kp, c0, c1))

            # Per-q accumulators and per-engine output tiles: separate tile
            # objects keep the framework's whole-tile dependency tracking
            # from serializing independent drain chains.
            tacc_q = {}
            stg_q = {}
            for bi, (q, c0, c1) in enumerate(blocks):
                tq = const.tile([64, c1 - c0], F32, tag=f"tacc{bi}", name=f"tacc{bi}")
                tacc_q[bi] = tq
                if c1 > cb:
                    # right-half staging: Pool cannot read PSUM, so Act lands
                    # the Ff-scaled psum there and Pool accumulates from SBUF
                    stg_q[bi] = [
                        const.tile([64, c1 - c0], F32, tag=f"stg{bi}_{p}",
                                   name=f"stg{bi}_{p}")
                        for p in range(2)
                    ]
            tout_a = const.tile([64, cb], BF16, tag="tout_a")
            tout_b = const.tile([64, BATCH - cb], BF16, tag="tout_b")
            zc_sb = const.tile([64, BATCH], BF16, tag="zc")
            # tout's complement regions (the unused row-half of each column)
            # are never written by the drain; zero them so the output DMA
            # reads defined values.
            nc.vector.memset(tout_a[:, :], 0.0)
            nc.gpsimd.memset(tout_b[:, :], 0.0)

            zc_ps = psum.tile([64, BATCH], F32, tag="zc")
            # T psum split per drain engine (columns 0:512 vs 512:1024) so
            # the two engines never touch the same psum tile: one bank each.
            t_ps = []
            for i in range(N_TPS):
                ta = psum.tile([64, cb], F32, tag=f"ta{i}", name=f"ta{i}")
                tb = psum.tile([64, BATCH - cb], F32, tag=f"tb{i}", name=f"tb{i}")
                t_ps.append((ta, tb))

            for kp in range(KPAIRS):
                # encoder: one accumulation chain over all 32 kps
                for c0 in range(0, BATCH, 256):
                    nc.tensor.matmul(
                        zc_ps[:, c0:c0 + 256],
                        wt[:, kp, :, :],
                        ft_rhs(kp, c0, c0 + 256),
                        start=(kp == 0), stop=(kp == KPAIRS - 1),
                        perf_mode=DR)
                # T: R12-only stationary; P_f accumulates over this f's 4 kps
                f, blk = divmod(kp, 4)
                tpa, tpb = t_ps[f % N_TPS]
                for (q, c0, c1) in blocks:
                    tp, ob = (tpa, 0) if c1 <= cb else (tpb, cb)
                    nc.tensor.matmul(
                        tp[0:64, c0 - ob:c1 - ob],
                        r12t[:, blk, :, 64 * q:64 * (q + 1)],
                        ft_rhs(kp, c0, c1),
                        start=(blk == 0), stop=(blk == 3),
                        perf_mode=DR)
                if blk == 3:
                    # fold in Ff: T(_acc/out) = Ff[f] * P_f (+ T_acc).
                    # Columns 0:512 drain on DVE straight from psum; for
                    # 512:1024 Act lands Ff*P_f into SBUF staging (gpsimd
                    # cannot read PSUM) and Pool accumulates from there.
                    for bi, (q, c0, c1) in enumerate(blocks):
                        rows = slice(0, 64)
                        left = c1 <= cb
                        ta = tacc_q[bi][rows, :]
                        if left:
                            sc = fft_v[rows, f, q:q + 1]
                            src = tpa[rows, c0:c1]
                            if f == 0:
                                nc.vector.tensor_scalar(ta, src, sc, None, MULT)
                            elif f < F_PER_CORE - 1:
                                nc.vector.scalar_tensor_tensor(
                                    ta, src, sc, ta, MULT, ADD)
                            else:
                                nc.vector.scalar_tensor_tensor(
                                    tout_a[rows, c0:c1], src, sc, ta, MULT, ADD)
                        else:
                            sc = fft_p[rows, f, q:q + 1]
                            src = tpb[rows, c0 - cb:c1 - cb]
                            if f == 0:
                                nc.scalar.mul(ta, src, sc)
                                continue
                            st = stg_q[bi][f % 2][rows, :]
                            nc.scalar.mul(st, src, sc)
                            if f < F_PER_CORE - 1:
                                nc.gpsimd.tensor_tensor(ta, st, ta, ADD)
                            else:
                                nc.gpsimd.tensor_tensor(
                                    tout_b[rows, c0 - cb:c1 - cb], st, ta, ADD)
            nc.scalar.copy(zc_sb[:, :], zc_ps[:, :])
            nc.scalar.dma_start(out=zcat_out[:, :], in_=zc_sb[:, :])
            nc.sync.dma_start(out=tt_out[:, 0:cb], in_=tout_a[:, :])
            nc.sync.dma_start(out=tt_out[:, cb:BATCH], in_=tout_b[:, :])
    return nc


def _prepare(inputs):
    features = np.asarray(inputs["features"], dtype=np.float32)
    labels = np.asarray(inputs["labels"]).astype(np.int64)
    groups = np.asarray(inputs["groups"]).astype(np.int64)
    weights = np.asarray(inputs["weights"], dtype=np.float32)
    noise = np.asarray(inputs["noise"], dtype=np.float32)
    group_embed = np.asarray(inputs["group_embed"], dtype=np.float32)
    W1 = np.asarray(inputs["W1"], dtype=np.float32)
    b1 = np.asarray(inputs["b1"], dtype=np.float32)
    W2 = np.asarray(inputs["W2"], dtype=np.float32)
    b2 = np.asarray(inputs["b2"], dtype=np.float32)
    freq_factors = np.asarray(inputs["freq_factors"], dtype=np.float32)
    roi_1_factors = np.asarray(inputs["roi_1_factors"], dtype=np.float32)
    roi_2_factors = np.asarray(inputs["roi_2_factors"], dtype=np.float32)
    lin_W = np.asarray(inputs["lin_W"], dtype=np.float32)
    lin_b = np.asarray(inputs["lin_b"], dtype=np.float32)
    logit_bias = np.asarray(inputs["logit_bias"], dtype=np.float32)

    b = features.shape[0]
    flat = features.reshape(b, -1)

    perm = np.argsort(groups, kind="stable")
    groups_sorted = groups[perm]
    blocks = _pair_blocks(groups_sorted)

    sq = np.einsum("bi,bi->b", flat, flat, optimize=True)

    flat_q = flat[perm].astype(NPFP8)
    flatT = flat_q.view(np.uint8).T.copy().view(NPFP8)      # [N_FEAT, BATCH]

    W = (np.concatenate([W1[:N_FEAT], W2[:N_FEAT]], axis=1) * W_SCALE).astype(NPFP8)

    Ff = _softplus(freq_factors)
    R1 = _softplus(roi_1_factors)
    R2 = _softplus(roi_2_factors)

    # R12[(r,s), (g,z)] in fp8 with a dynamic power-of-two scale.
    R12 = (R1[:, :, :, None] * R2[:, :, None, :])           # [g, z, r, s]
    # ml_dtypes.float8_e4m3 is the IEEE variant: max finite value is 240
    r_scale = float(2.0 ** np.floor(np.log2(224.0 / max(float(R12.max()), 1e-6))))
    R12q = (R12 * r_scale).transpose(2, 3, 0, 1).reshape(RS, N_GROUPS * Z)
    R12q = R12q.astype(NPFP8)                               # [rs, gz]
    r12_dev = R12q.view(np.uint8).reshape(4, 2, 128, N_GROUPS * Z)
    r12_dev = r12_dev.transpose(2, 0, 1, 3).copy().view(NPFP8)

    # ff[i, f, q] = Ff[g, z, f]/r_scale with g = 2q+i//32, z = i%32 (i < 64)
    ff_all = np.zeros((N_CORES, 128, F_PER_CORE, 8), np.float32)
    for q in range(N_GROUPS // 2):
        for gp in range(2):
            g = 2 * q + gp
            p0 = gp * 32
            for c in range(N_CORES):
                fglob = np.arange(F_PER_CORE) + c * F_PER_CORE
                ff_all[c, p0:p0 + 32, :, q] = Ff[g, :, fglob].T / r_scale

    in_maps = []
    for c in range(N_CORES):
        fsl = flatT[c * FEAT_PER_CORE:(c + 1) * FEAT_PER_CORE]
        fsl = fsl.view(np.uint8).reshape(KPAIRS, 2, 128, BATCH)
        fsl = fsl.transpose(2, 0, 1, 3).copy().view(NPFP8)
        wsl = W[c * FEAT_PER_CORE:(c + 1) * FEAT_PER_CORE].view(np.uint8)
        wsl = wsl.reshape(KPAIRS, 2, 128, 64).transpose(2, 0, 1, 3).copy().view(NPFP8)
        in_maps.append({
            "flatt": fsl,
            "w": wsl,
            "r12": r12_dev,
            "ff": ff_all[c],
        })

    host = dict(
        labels=labels, groups=groups, weights=weights, noise=noise,
        group_embed=group_embed, W1=W1, b1=b1, W2=W2, b2=b2,
        lin_W=lin_W, lin_b=lin_b, logit_bias=logit_bias,
        Ff=Ff, R1=R1, R2=R2, sq=sq, perm=perm, b=b,
        groups_sorted=groups_sorted,
    )
    return in_maps, blocks, host


def _finish(zcatT, ttT, host):
    b = host["b"]
    perm = host["perm"]
    inv = np.empty_like(perm)
    inv[perm] = np.arange(b)

    zcat = (zcatT / W_SCALE).T[inv]                        # [b, 64]
    gs2 = (host["groups_sorted"] % 2).astype(np.int64)     # row half per sorted col
    cols = np.arange(b)
    T_sorted = np.empty((b, Z), np.float32)
    for zi in range(Z):
        T_sorted[:, zi] = ttT[gs2 * Z + zi, cols]
    T = T_sorted[inv]

    groups = host["groups"]
    ge = host["group_embed"][groups]
    z_mu = zcat[:, :Z] + host["b1"] + ge @ host["W1"][N_FEAT:]
    z_log_std = zcat[:, Z:] + host["b2"] + ge @ host["W2"][N_FEAT:]
    sigma = EPSILON + np.exp(z_log_std)
    kld = np.sum(-np.log(sigma) + 0.5 * (sigma * sigma + z_mu * z_mu - 1.0), axis=1)
    zs = z_mu + sigma * host["noise"]
    zs = zs @ host["lin_W"] + host["lin_b"]
    a = _softplus(zs)

    Ff, R1, R2 = host["Ff"], host["R1"], host["R2"]
    M = (np.einsum("gzf,gyf->gzy", Ff, Ff)
         * np.einsum("gzr,gyr->gzy", R1, R1)
         * np.einsum("gzs,gys->gzy", R2, R2))
    vol2 = np.einsum("bz,bzy,by->b", a, M[groups], a)
    fdotv = np.sum(a * T, axis=1)
    rec = REG_STRENGTH * (host["sq"] - 2.0 * fdotv + vol2) / N_FEAT

    logits = np.concatenate([zs[:, :N_CLASSES - 1], np.ones((b, 1), np.float32)],
                            axis=1) + host["logit_bias"]
    m = logits.max(axis=1, keepdims=True)
    lse = m[:, 0] + np.log(np.exp(logits - m).sum(axis=1))
    log_probs = logits[np.arange(b), host["labels"]] - lse

    freq_loss = np.var(Ff, axis=0, ddof=1).mean(axis=1).sum()
    roi_loss = (np.var(R1, axis=0, ddof=1) + np.var(R2, axis=0, ddof=1)).mean(axis=1).sum()

    loss = np.mean(rec - host["weights"] * log_probs + KL_FACTOR * kld) \
        + freq_loss + roi_loss
    return np.float32(loss)


def kernel(**inputs) -> np.ndarray:
    _install_waitfix()
    in_maps, blocks, host = _prepare(inputs)
    nc = build_device_program(blocks)
    r = run_bass_kernel_spmd(nc, in_maps, core_ids=list(range(N_CORES)))
    zcatT = np.zeros((64, BATCH), np.float32)
    ttT = np.zeros((64, BATCH), np.float32)
    for c in range(N_CORES):
        zcatT += r.results[c]["zcat"].astype(np.float32)
        ttT += r.results[c]["tt"].astype(np.float32)
    return _finish(zcatT, ttT, host)
